# revision 12
# baseline (speedup 1.0000x reference)
"""AttnDecoderRNN on 8 trn2 NeuronCores.

Strategy: shard encoder S=8192 across 8 cores (1024 each). All loop-invariant
weights live SBUF-resident in bf16. Per decode step:
  - qW = h@Wa (TensorE, vec-as-weights trick), tanh(qW+kU) on ScalarE with
    per-partition bias, e = Va-reduction (TensorE), exp (ScalarE),
    ctx~ = w~ @ keys (TensorE)  -- all on the local S-shard.
  - ONE AllReduce of [ctx~ (1024); Z_part] ; GRU gh/gi_emb matmuls overlap it.
  - GRU gates + EOS logic computed redundantly on every core (tiny).
  - ONE AllGather of the core's h-shard (128) -> full h everywhere.
Attention weights are stored unnormalized (bf16) + Z per step; the host
divides during unshard.
"""

import sys
import numpy as np

sys.path.insert(0, "/opt/trn_rl_repo")

import ml_dtypes

H = 1024
S = 8192
V = 5
T = 23
NCORES = 8
SS = S // NCORES  # 1024
BF = ml_dtypes.bfloat16

_CACHE = {}


def _build(sim=False):
    from concourse import bass, bacc, tile, mybir
    from contextlib import ExitStack

    f32 = mybir.dt.float32
    bf16 = mybir.dt.bfloat16
    AF = mybir.ActivationFunctionType
    ALU = mybir.AluOpType
    AX = mybir.AxisListType

    nc = bacc.Bacc(
        "TRN2",
        target_bir_lowering=False,
        debug=False,
        enable_asserts=True,
        num_devices=1 if sim else NCORES,
    )

    # ---- per-core DRAM inputs -------------------------------------------
    d_keys = nc.dram_tensor("keys", [SS, H], bf16, kind="ExternalInput")
    d_keysT = nc.dram_tensor("keysT", [H + 1, SS], bf16, kind="ExternalInput")
    d_ua = nc.dram_tensor("ua", [H + 1, H], bf16, kind="ExternalInput")
    d_wa = nc.dram_tensor("wa", [H + 1, H], bf16, kind="ExternalInput")
    d_wl = nc.dram_tensor("wl", [V + 1, H], bf16, kind="ExternalInput")
    d_wih = nc.dram_tensor("wih", [2 * H + 1, 384], bf16, kind="ExternalInput")
    d_whh = nc.dram_tensor("whh", [H + 1, 384], bf16, kind="ExternalInput")
    d_wout = nc.dram_tensor("wout", [H + 1, V], f32, kind="ExternalInput")
    d_va = nc.dram_tensor("va", [128, 8], bf16, kind="ExternalInput")
    d_h0 = nc.dram_tensor("h0", [8, 128], f32, kind="ExternalInput")
    d_hmy0 = nc.dram_tensor("hmy0", [1, 128], f32, kind="ExternalInput")
    d_id8f = nc.dram_tensor("id8f", [8, 8], f32, kind="ExternalInput")
    d_id8b = nc.dram_tensor("id8b", [8, 8], bf16, kind="ExternalInput")
    d_id1f = nc.dram_tensor("id1f", [1, 1], f32, kind="ExternalInput")
    d_eos = nc.dram_tensor("eos", [1, V], f32, kind="ExternalInput")
    d_bv8 = nc.dram_tensor("bv8", [8, 1], f32, kind="ExternalInput")

    # ---- per-core DRAM outputs ------------------------------------------
    d_outs = nc.dram_tensor("outs", [T, V], f32, kind="ExternalOutput")
    d_hfin = nc.dram_tensor("hfin", [8, 128], f32, kind="ExternalOutput")
    d_attns = nc.dram_tensor("attns", [T, 8, 128], bf16, kind="ExternalOutput")
    d_zout = nc.dram_tensor("zout", [T, 1], f32, kind="ExternalOutput")

    RG = [list(range(NCORES))]

    with tile.TileContext(nc) as tc, ExitStack() as ctx:
        wpool = ctx.enter_context(tc.tile_pool(name="w", bufs=1))
        state = ctx.enter_context(tc.tile_pool(name="st", bufs=1))
        work = ctx.enter_context(tc.tile_pool(name="wk", bufs=3))
        big = ctx.enter_context(tc.tile_pool(name="big", bufs=1))
        ps8 = ctx.enter_context(tc.tile_pool(name="p8", bufs=2, space="PSUM"))
        psv = ctx.enter_context(tc.tile_pool(name="pv", bufs=4, space="PSUM"))
        dram = ctx.enter_context(tc.tile_pool(name="dr", bufs=3, space="DRAM"))

        # ---------------- resident weights ----------------
        keys_sb = wpool.tile([128, 8 * H], bf16, tag="keys")
        wa_sb = wpool.tile([128, 8 * H], bf16, tag="wa")
        wab_sb = wpool.tile([1, H], bf16, tag="wab")
        wih_sb = wpool.tile([128, 16 * 384], bf16, tag="wih")
        wihb_sb = wpool.tile([1, 384], bf16, tag="wihb")
        whh_sb = wpool.tile([128, 8 * 384], bf16, tag="whh")
        whhb_sb = wpool.tile([1, 384], bf16, tag="whhb")
        wout_sb = wpool.tile([128, 8 * V], f32, tag="wout")
        woutb_sb = wpool.tile([1, V], f32, tag="woutb")
        wl_sb = wpool.tile([V + 1, H], bf16, tag="wl")
        va_sb = wpool.tile([128, 8], bf16, tag="va")
        id8f_sb = wpool.tile([8, 8], f32, tag="id8f")
        id8b_sb = wpool.tile([8, 8], bf16, tag="id8b")
        id1f_sb = wpool.tile([1, 1], f32, tag="id1f")
        eos_sb = wpool.tile([1, V], f32, tag="eos")
        bv8_sb = wpool.tile([8, 1], f32, tag="bv8")
        ones8b_sb = wpool.tile([8, 1], bf16, tag="ones8b")
        ones128_sb = wpool.tile([1, 128], f32, tag="ones128")
        one_bf = wpool.tile([1, 1], bf16, tag="onebf")
        one_f = wpool.tile([1, 1], f32, tag="onef")
        kut_sb = wpool.tile([128, 8 * SS], bf16, tag="kut")

        for r in range(8):
            nc.sync.dma_start(
                keys_sb[:, r * H : (r + 1) * H], d_keys.ap()[r * 128 : (r + 1) * 128, :]
            )
            nc.sync.dma_start(
                wa_sb[:, r * H : (r + 1) * H], d_wa.ap()[r * 128 : (r + 1) * 128, :]
            )
            nc.sync.dma_start(
                whh_sb[:, r * 384 : (r + 1) * 384],
                d_whh.ap()[r * 128 : (r + 1) * 128, :],
            )
            nc.sync.dma_start(
                wout_sb[:, r * V : (r + 1) * V], d_wout.ap()[r * 128 : (r + 1) * 128, :]
            )
        for c in range(16):
            nc.sync.dma_start(
                wih_sb[:, c * 384 : (c + 1) * 384],
                d_wih.ap()[c * 128 : (c + 1) * 128, :],
            )
        nc.sync.dma_start(wab_sb[:], d_wa.ap()[H : H + 1, :])
        nc.sync.dma_start(wihb_sb[:], d_wih.ap()[2 * H : 2 * H + 1, :])
        nc.sync.dma_start(whhb_sb[:], d_whh.ap()[H : H + 1, :])
        nc.sync.dma_start(woutb_sb[:], d_wout.ap()[H : H + 1, :])
        nc.sync.dma_start(wl_sb[:], d_wl.ap()[:, :])
        nc.sync.dma_start(va_sb[:], d_va.ap()[:, :])
        nc.sync.dma_start(id8f_sb[:], d_id8f.ap()[:, :])
        nc.sync.dma_start(id8b_sb[:], d_id8b.ap()[:, :])
        nc.sync.dma_start(id1f_sb[:], d_id1f.ap()[:, :])
        nc.sync.dma_start(eos_sb[:], d_eos.ap()[:, :])
        nc.sync.dma_start(bv8_sb[:], d_bv8.ap()[:, :])
        nc.vector.memset(ones8b_sb[:], 1.0)
        nc.vector.memset(ones128_sb[:], 1.0)
        nc.vector.memset(one_bf[:], 1.0)
        nc.vector.memset(one_f[:], 1.0)

        # ---------------- kUT precompute ----------------
        with tc.tile_pool(name="pre", bufs=1) as pre:
            kt_sb = pre.tile([128, 9 * SS], bf16, tag="kt")
            ua_sb = pre.tile([128, 9 * H], bf16, tag="uat")
            for c in range(9):
                rows = 128 if c < 8 else 1
                nc.sync.dma_start(
                    kt_sb[0:rows, c * SS : c * SS + SS],
                    d_keysT.ap()[c * 128 : c * 128 + rows, :],
                )
                nc.sync.dma_start(
                    ua_sb[0:rows, c * H : c * H + H],
                    d_ua.ap()[c * 128 : c * 128 + rows, :],
                )
            for ht in range(8):
                for nb in range(2):
                    pm = ps8.tile([128, 512], mybir.dt.float32, tag="t8")
                    for c in range(9):
                        rows = 128 if c < 8 else 1
                        nc.tensor.matmul(
                            pm[:],
                            ua_sb[0:rows, c * H + ht * 128 : c * H + (ht + 1) * 128],
                            kt_sb[0:rows, c * SS + nb * 512 : c * SS + nb * 512 + 512],
                            start=(c == 0),
                            stop=(c == 8),
                        )
                    nc.scalar.activation(
                        kut_sb[:, ht * SS + nb * 512 : ht * SS + nb * 512 + 512],
                        pm[:],
                        AF.Copy,
                    )

        # ---------------- persistent state ----------------
        h_free = state.tile([8, 128], f32, tag="hfree")
        h_par_bf = state.tile([128, 8], bf16, tag="hparb")
        h_par_f = state.tile([128, 8], f32, tag="hparf")
        h_my = state.tile([1, 128], f32, tag="hmy")
        inp_par = state.tile([V + 1, 1], bf16, tag="inppar")
        inp_f6 = state.tile([1, V + 1], f32, tag="inpf")
        done1 = state.tile([1, 1], f32, tag="done1")
        done5 = state.tile([1, V], f32, tag="done5")
        done128 = state.tile([1, 128], f32, tag="done128")

        nc.sync.dma_start(h_free[:], d_h0.ap()[:, :])
        nc.sync.dma_start(h_my[:], d_hmy0.ap()[:, :])
        hi_ps = ps8.tile([128, 8], f32, tag="t8")
        nc.tensor.transpose(hi_ps[:], h_free[:], id8f_sb[:])
        nc.vector.tensor_copy(h_par_bf[:], hi_ps[:])
        nc.vector.tensor_copy(h_par_f[:], hi_ps[:])
        nc.vector.memset(inp_f6[0:1, 0:V], 0.0)
        nc.vector.memset(inp_f6[0:1, V : V + 1], 1.0)
        ip0_ps = psv.tile([V + 1, 1], f32, tag="vec")
        nc.tensor.transpose(ip0_ps[:], inp_f6[:], id1f_sb[:])
        nc.vector.tensor_copy(inp_par[:], ip0_ps[:])
        nc.vector.memset(done1[:], 0.0)
        nc.vector.memset(done5[:], 0.0)
        nc.vector.memset(done128[:], 0.0)

        # ---------------- decode loop ----------------
        for t in range(T):
            # emb in par layout (128, 8) += bias via aug row of Wl
            emb_ps = ps8.tile([128, 8], f32, tag="t8")
            for ht in range(8):
                nc.tensor.matmul(
                    emb_ps[:, ht : ht + 1],
                    wl_sb[:, ht * 128 : (ht + 1) * 128],
                    inp_par[:, 0:1],
                    start=True,
                    stop=True,
                )
            emb_bf = work.tile([128, 8], bf16, tag="embbf")
            nc.vector.tensor_copy(emb_bf[:], emb_ps[:])

            # qW = h @ Wa + ba  -> free layout (1,1024) in two psum halves
            qw_ps = []
            for nb in range(2):
                pm = psv.tile([1, 512], f32, tag="vec")
                for c in range(8):
                    nc.tensor.matmul(
                        pm[:],
                        h_par_bf[:, c : c + 1],
                        wa_sb[:, c * H + nb * 512 : c * H + nb * 512 + 512],
                        start=(c == 0),
                        stop=False,
                    )
                nc.tensor.matmul(
                    pm[:],
                    one_bf[:],
                    wab_sb[:, nb * 512 : nb * 512 + 512],
                    start=False,
                    stop=True,
                )
                qw_ps.append(pm)
            qwf = work.tile([1, H], f32, tag="qwf")
            nc.vector.tensor_copy(qwf[0:1, 0:512], qw_ps[0][:])
            nc.vector.tensor_copy(qwf[0:1, 512:1024], qw_ps[1][:])
            qw8 = work.tile([8, 128], f32, tag="qw8")
            nc.sync.dma_start(qw8[:], qwf[:])
            qwt_ps = ps8.tile([128, 8], f32, tag="t8")
            nc.tensor.transpose(qwt_ps[:], qw8[:], id8f_sb[:])
            qw_par = work.tile([128, 8], f32, tag="qwpar")
            nc.vector.tensor_copy(qw_par[:], qwt_ps[:])

            # tanh(qW + kU) on ScalarE, per h-tile, bias = qW column
            th = big.tile([128, 8 * SS], bf16, tag="th")
            for ht in range(8):
                nc.scalar.activation(
                    th[:, ht * SS : (ht + 1) * SS],
                    kut_sb[:, ht * SS : (ht + 1) * SS],
                    AF.Tanh,
                    bias=qw_par[:, ht : ht + 1],
                )

            # e = Va . tanh  (accumulate over h tiles)
            e_ps = []
            for nb in range(2):
                pm = psv.tile([1, 512], f32, tag="vec")
                for ht in range(8):
                    nc.tensor.matmul(
                        pm[:],
                        va_sb[:, ht : ht + 1],
                        th[:, ht * SS + nb * 512 : ht * SS + nb * 512 + 512],
                        start=(ht == 0),
                        stop=(ht == 7),
                    )
                e_ps.append(pm)
            ef = work.tile([1, H], f32, tag="ef")
            nc.vector.tensor_copy(ef[0:1, 0:512], e_ps[0][:])
            nc.vector.tensor_copy(ef[0:1, 512:1024], e_ps[1][:])
            e8 = work.tile([8, 128], f32, tag="e8")
            nc.sync.dma_start(e8[:], ef[:])

            # w~ = exp(e + bv)
            w8 = work.tile([8, 128], bf16, tag="w8")
            nc.scalar.activation(w8[:], e8[:], AF.Exp, bias=bv8_sb[:])
            nc.gpsimd.dma_start(d_attns.ap()[t], w8[:])
            zr_ps = psv.tile([1, 128], f32, tag="vec")
            nc.tensor.matmul(zr_ps[:], ones8b_sb[:], w8[:], start=True, stop=True)

            wt_ps = ps8.tile([128, 8], bf16, tag="t8")
            nc.tensor.transpose(wt_ps[:], w8[:], id8b_sb[:])
            w_par = work.tile([128, 8], bf16, tag="wpar")
            nc.vector.tensor_copy(w_par[:], wt_ps[:])

            # ctx~ = w~ @ keys  (unnormalized)
            c_ps = []
            for nb in range(2):
                pm = psv.tile([1, 512], f32, tag="vec")
                for r in range(8):
                    nc.tensor.matmul(
                        pm[:],
                        w_par[:, r : r + 1],
                        keys_sb[:, r * H + nb * 512 : r * H + nb * 512 + 512],
                        start=(r == 0),
                        stop=(r == 7),
                    )
                c_ps.append(pm)

            # AllReduce [ctx~ ; Z]
            arin = dram.tile([1, H + 1], f32, tag="arin")
            arout = dram.tile([1, H + 1], f32, tag="arout")
            cf = work.tile([1, H + 1], f32, tag="cf")
            nc.vector.tensor_copy(cf[0:1, 0:512], c_ps[0][:])
            nc.vector.tensor_copy(cf[0:1, 512:1024], c_ps[1][:])
            nc.vector.reduce_sum(cf[0:1, 1024 : H + 1], zr_ps[:], axis=AX.X)
            nc.sync.dma_start(arin[0:1, 0 : H + 1], cf[:])
            if sim:
                nc.sync.dma_start(arout[:], arin[:])
            else:
                nc.gpsimd.collective_compute(
                    "AllReduce",
                    mybir.AluOpType.add,
                    replica_groups=RG,
                    ins=[arin.opt()],
                    outs=[arout.opt()],
                )

            # overlap with AR: gh = W_hh @ h + b_hh ; gi_emb = W_ih[:, :H] @ emb + b_ih
            gh_ps = psv.tile([1, 384], f32, tag="vec")
            for c in range(8):
                nc.tensor.matmul(
                    gh_ps[:],
                    h_par_bf[:, c : c + 1],
                    whh_sb[:, c * 384 : (c + 1) * 384],
                    start=(c == 0),
                    stop=False,
                )
            nc.tensor.matmul(gh_ps[:], one_bf[:], whhb_sb[:], start=False, stop=True)
            gie_ps = psv.tile([1, 384], f32, tag="vec")
            for c in range(8):
                nc.tensor.matmul(
                    gie_ps[:],
                    emb_bf[:, c : c + 1],
                    wih_sb[:, c * 384 : (c + 1) * 384],
                    start=(c == 0),
                    stop=False,
                )
            nc.tensor.matmul(gie_ps[:], one_bf[:], wihb_sb[:], start=False, stop=True)

            # post-AR: ctx, 1/Z
            ctx8 = work.tile([8, 128], f32, tag="ctx8")
            nc.sync.dma_start(ctx8[:], arout[0:1, 0:1024])
            zg = work.tile([1, 1], f32, tag="zg")
            nc.sync.dma_start(zg[:], arout[0:1, 1024 : H + 1])
            nc.gpsimd.dma_start(d_zout.ap()[t], arout[0:1, 1024 : H + 1])
            prec = work.tile([1, 1], f32, tag="prec")
            nc.vector.reciprocal(prec[:], zg[:])
            ctxt_ps = ps8.tile([128, 8], f32, tag="t8")
            nc.tensor.transpose(ctxt_ps[:], ctx8[:], id8f_sb[:])
            ctx_par = work.tile([128, 8], bf16, tag="ctxpar")
            nc.vector.tensor_copy(ctx_par[:], ctxt_ps[:])

            gic_ps = psv.tile([1, 384], f32, tag="vec")
            for c in range(8):
                nc.tensor.matmul(
                    gic_ps[:],
                    ctx_par[:, c : c + 1],
                    wih_sb[:, (8 + c) * 384 : (9 + c) * 384],
                    start=(c == 0),
                    stop=(c == 7),
                )

            # gates (free layout, partition 0)
            gi = work.tile([1, 384], f32, tag="gi")
            nc.vector.tensor_scalar_mul(gi[:], gic_ps[:], prec[:])
            nc.vector.tensor_add(gi[:], gi[:], gie_ps[:])
            rz_in = work.tile([1, 256], f32, tag="rzin")
            nc.vector.tensor_add(rz_in[:], gi[0:1, 0:256], gh_ps[0:1, 0:256])
            rz = work.tile([1, 256], f32, tag="rz")
            nc.scalar.activation(rz[:], rz_in[:], AF.Tanh, scale=0.5)
            nc.vector.tensor_scalar(rz[:], rz[:], 0.5, 0.5, ALU.mult, ALU.add)
            nin = work.tile([1, 128], f32, tag="nin")
            nc.vector.tensor_mul(nin[:], rz[0:1, 0:128], gh_ps[0:1, 256:384])
            nc.vector.tensor_add(nin[:], nin[:], gi[0:1, 256:384])
            nn = work.tile([1, 128], f32, tag="nn")
            nc.scalar.activation(nn[:], nin[:], AF.Tanh)
            d1 = work.tile([1, 128], f32, tag="d1")
            nc.vector.tensor_sub(d1[:], h_my[:], nn[:])
            nc.vector.tensor_mul(d1[:], d1[:], rz[0:1, 128:256])
            hn = work.tile([1, 128], f32, tag="hn")
            nc.vector.tensor_add(hn[:], nn[:], d1[:])
            # freeze h after done (uses done from PREVIOUS step)
            d2 = work.tile([1, 128], f32, tag="d2")
            nc.vector.tensor_sub(d2[:], h_my[:], hn[:])
            nc.vector.tensor_mul(d2[:], d2[:], done128[:])
            nc.vector.tensor_add(h_my[:], hn[:], d2[:])

            # AllGather h
            agin = dram.tile([1, 128], f32, tag="agin")
            agout = dram.tile([8, 128], f32, tag="agout")
            nc.sync.dma_start(agin[:], h_my[:])
            if sim:
                nc.sync.dma_start(agout[0:1, :], agin[:])
            else:
                nc.gpsimd.collective_compute(
                    "AllGather",
                    mybir.AluOpType.bypass,
                    replica_groups=RG,
                    ins=[agin.opt()],
                    outs=[agout.opt()],
                )
            nc.sync.dma_start(h_free[:], agout[:])
            ht_ps = ps8.tile([128, 8], f32, tag="t8")
            nc.tensor.transpose(ht_ps[:], h_free[:], id8f_sb[:])
            nc.vector.tensor_copy(h_par_bf[:], ht_ps[:])
            nc.vector.tensor_copy(h_par_f[:], ht_ps[:])

            # logits (f32 path to keep the EOS decision faithful)
            lg_ps = psv.tile([1, V], f32, tag="vec")
            for c in range(8):
                nc.tensor.matmul(
                    lg_ps[:],
                    h_par_f[:, c : c + 1],
                    wout_sb[:, c * V : (c + 1) * V],
                    start=(c == 0),
                    stop=False,
                )
            nc.tensor.matmul(lg_ps[:], one_f[:], woutb_sb[:], start=False, stop=True)
            pexp = work.tile([1, V], f32, tag="pexp")
            nc.scalar.activation(pexp[:], lg_ps[:], AF.Exp)
            psm = work.tile([1, 1], f32, tag="psm")
            nc.vector.reduce_sum(psm[:], pexp[:], axis=AX.X)
            pr = work.tile([1, 1], f32, tag="pr")
            nc.vector.reciprocal(pr[:], psm[:])
            probs = work.tile([1, V], f32, tag="probs")
            nc.vector.tensor_scalar_mul(probs[:], pexp[:], pr[:])
            if t >= 5:
                pmax = work.tile([1, 1], f32, tag="pmax")
                nc.vector.reduce_max(pmax[:], pexp[:], axis=AX.X)
                trig = work.tile([1, 1], f32, tag="trig")
                nc.vector.tensor_tensor(
                    trig[:], pexp[0:1, V - 1 : V], pmax[:], op=ALU.is_ge
                )
                nc.vector.tensor_max(done1[:], done1[:], trig[:])
                nc.vector.tensor_scalar_mul(done128[:], ones128_sb[:], done1[:])
                nc.vector.tensor_scalar_mul(done5[:], ones128_sb[0:1, 0:V], done1[:])

            # out_t = probs + done*(EOS - probs) ; inp = probs + done*(inp - probs)
            o1 = work.tile([1, V], f32, tag="o1")
            nc.vector.tensor_sub(o1[:], eos_sb[:], probs[:])
            nc.vector.tensor_mul(o1[:], o1[:], done5[:])
            nc.vector.tensor_add(o1[:], o1[:], probs[:])
            nc.gpsimd.dma_start(d_outs.ap()[t], o1[:])
            i1 = work.tile([1, V], f32, tag="i1")
            nc.vector.tensor_sub(i1[:], inp_f6[0:1, 0:V], probs[:])
            nc.vector.tensor_mul(i1[:], i1[:], done5[:])
            nc.vector.tensor_add(inp_f6[0:1, 0:V], probs[:], i1[:])
            ip_ps = psv.tile([V + 1, 1], f32, tag="vec")
            nc.tensor.transpose(ip_ps[:], inp_f6[:], id1f_sb[:])
            nc.vector.tensor_copy(inp_par[:], ip_ps[:])

        nc.sync.dma_start(d_hfin.ap()[:, :], h_free[:])

    nc.compile()
    return nc


def _get_nc():
    if "nc" not in _CACHE:
        _CACHE["nc"] = _build()
    return _CACHE["nc"]


def kernel(**inputs):
    from concourse.bass_utils import run_bass_kernel_spmd

    f32 = np.float32
    keys = np.asarray(inputs["encoder_outputs"], f32)[0]  # (S, H)
    h0 = np.asarray(inputs["encoder_hidden"], f32)[0, 0]  # (H,)
    Wa = np.asarray(inputs["Wa"], f32)
    ba = np.asarray(inputs["ba"], f32)
    Ua = np.asarray(inputs["Ua"], f32)
    bu = np.asarray(inputs["bu"], f32)
    Va = np.asarray(inputs["Va"], f32)
    bv = np.asarray(inputs["bv"], f32)
    Wl = np.asarray(inputs["Wl"], f32)
    bl = np.asarray(inputs["bl"], f32)
    W_ih = np.asarray(inputs["W_ih"], f32)
    b_ih = np.asarray(inputs["b_ih"], f32)
    W_hh = np.asarray(inputs["W_hh"], f32)
    b_hh = np.asarray(inputs["b_hh"], f32)
    W_out = np.asarray(inputs["W_out"], f32)
    b_out = np.asarray(inputs["b_out"], f32)

    Ua_aug = np.vstack([Ua, bu[None, :]]).astype(BF)
    Wa_aug = np.vstack([Wa, ba[None, :]]).astype(BF)
    Wl_aug = np.vstack([Wl, bl[None, :]]).astype(BF)
    Wout_aug = np.vstack([W_out, b_out[None, :]]).astype(f32)
    va_par = np.ascontiguousarray(Va[:, 0].reshape(8, 128).T).astype(BF)

    shared = {
        "ua": Ua_aug,
        "wa": Wa_aug,
        "wl": Wl_aug,
        "wout": Wout_aug,
        "va": va_par,
        "h0": np.ascontiguousarray(h0.reshape(8, 128)),
        "id8f": np.eye(8, dtype=f32),
        "id8b": np.eye(8).astype(BF),
        "id1f": np.ones((1, 1), f32),
        "eos": np.array([[0, 0, 0, 0, 1]], f32),
        "bv8": np.full((8, 1), float(bv[0]), f32),
    }
    in_maps = []
    for k in range(NCORES):
        sl = slice(k * SS, (k + 1) * SS)
        keys_k = keys[sl]
        keysT_aug = np.vstack([keys_k.T, np.ones((1, SS), f32)]).astype(BF)
        rows = np.r_[
            k * 128 : (k + 1) * 128,
            H + k * 128 : H + (k + 1) * 128,
            2 * H + k * 128 : 2 * H + (k + 1) * 128,
        ]
        WihT_aug = np.vstack([W_ih[rows, :].T, b_ih[rows][None, :]]).astype(BF)
        WhhT_aug = np.vstack([W_hh[rows, :].T, b_hh[rows][None, :]]).astype(BF)
        m = dict(shared)
        m.update(
            {
                "keys": keys_k.astype(BF),
                "keysT": keysT_aug,
                "wih": WihT_aug,
                "whh": WhhT_aug,
                "hmy0": np.ascontiguousarray(h0[k * 128 : (k + 1) * 128][None, :]),
            }
        )
        in_maps.append(m)

    nc = _get_nc()
    res = run_bass_kernel_spmd(nc, in_maps, core_ids=list(range(NCORES)))
    r = res.results
    outs = np.asarray(r[0]["outs"], f32).reshape(T, V)
    hfin = np.asarray(r[0]["hfin"], f32).reshape(H)
    Z = np.asarray(r[0]["zout"], f32).reshape(T)
    att = np.concatenate(
        [np.asarray(r[k]["attns"]).astype(f32).reshape(T, SS) for k in range(NCORES)],
        axis=1,
    )
    attns = att / Z[:, None]
    return outs[None], hfin[None, None], attns[None]


if __name__ == "__main__":
    _get_nc()
    print("build+compile OK")


# revision 14
# speedup vs baseline: 1.0029x; 1.0029x over previous
"""AttnDecoderRNN on 8 trn2 NeuronCores.

Strategy: shard encoder S=8192 across 8 cores (1024 each). All loop-invariant
weights live SBUF-resident in bf16. Per decode step:
  - qW = h@Wa (TensorE, vec-as-weights trick), tanh(qW+kU) on ScalarE with
    per-partition bias, e = Va-reduction (TensorE), exp (ScalarE),
    ctx~ = w~ @ keys (TensorE)  -- all on the local S-shard.
  - ONE AllReduce of [ctx~ (1024); Z_part] ; GRU gh/gi_emb matmuls overlap it.
  - GRU gates + EOS logic computed redundantly on every core (tiny).
  - ONE AllGather of the core's h-shard (128) -> full h everywhere.
Attention weights are stored unnormalized (bf16) + Z per step; the host
divides during unshard.
"""

import sys
import numpy as np

sys.path.insert(0, "/opt/trn_rl_repo")

import ml_dtypes

H = 1024
S = 8192
V = 5
T = 23
NCORES = 8
SS = S // NCORES  # 1024
BF = ml_dtypes.bfloat16

_CACHE = {}


def _build(sim=False):
    from concourse import bass, bacc, tile, mybir
    from contextlib import ExitStack

    f32 = mybir.dt.float32
    bf16 = mybir.dt.bfloat16
    AF = mybir.ActivationFunctionType
    ALU = mybir.AluOpType
    AX = mybir.AxisListType

    nc = bacc.Bacc(
        "TRN2",
        target_bir_lowering=False,
        debug=False,
        enable_asserts=True,
        num_devices=1 if sim else NCORES,
    )

    # ---- per-core DRAM inputs -------------------------------------------
    d_keys = nc.dram_tensor("keys", [SS, H], bf16, kind="ExternalInput")
    d_keysT = nc.dram_tensor("keysT", [H + 1, SS], bf16, kind="ExternalInput")
    d_ua = nc.dram_tensor("ua", [H + 1, H], bf16, kind="ExternalInput")
    d_wa = nc.dram_tensor("wa", [H + 1, H], bf16, kind="ExternalInput")
    d_wl = nc.dram_tensor("wl", [V + 1, H], bf16, kind="ExternalInput")
    d_wih = nc.dram_tensor("wih", [2 * H + 1, 384], bf16, kind="ExternalInput")
    d_whh = nc.dram_tensor("whh", [H + 1, 384], bf16, kind="ExternalInput")
    d_wout = nc.dram_tensor("wout", [H + 1, V], f32, kind="ExternalInput")
    d_va = nc.dram_tensor("va", [128, 8], bf16, kind="ExternalInput")
    d_h0 = nc.dram_tensor("h0", [8, 128], f32, kind="ExternalInput")
    d_hmy0 = nc.dram_tensor("hmy0", [1, 128], f32, kind="ExternalInput")
    d_id8f = nc.dram_tensor("id8f", [8, 8], f32, kind="ExternalInput")
    d_id8b = nc.dram_tensor("id8b", [8, 8], bf16, kind="ExternalInput")
    d_id1f = nc.dram_tensor("id1f", [1, 1], f32, kind="ExternalInput")
    d_eos = nc.dram_tensor("eos", [1, V], f32, kind="ExternalInput")
    d_bv8 = nc.dram_tensor("bv8", [8, 1], f32, kind="ExternalInput")
    d_bv1 = nc.dram_tensor("bv1", [1, 1], f32, kind="ExternalInput")

    # ---- per-core DRAM outputs ------------------------------------------
    d_outs = nc.dram_tensor("outs", [T, V], f32, kind="ExternalOutput")
    d_hfin = nc.dram_tensor("hfin", [8, 128], f32, kind="ExternalOutput")
    d_attns = nc.dram_tensor("attns", [T, 8, 128], bf16, kind="ExternalOutput")
    d_zout = nc.dram_tensor("zout", [T, 1], f32, kind="ExternalOutput")

    RG = [list(range(NCORES))]

    with tile.TileContext(nc) as tc, ExitStack() as ctx:
        wpool = ctx.enter_context(tc.tile_pool(name="w", bufs=1))
        state = ctx.enter_context(tc.tile_pool(name="st", bufs=1))
        work = ctx.enter_context(tc.tile_pool(name="wk", bufs=3))
        big = ctx.enter_context(tc.tile_pool(name="big", bufs=1))
        ps8 = ctx.enter_context(tc.tile_pool(name="p8", bufs=2, space="PSUM"))
        psv = ctx.enter_context(tc.tile_pool(name="pv", bufs=4, space="PSUM"))
        dram = ctx.enter_context(tc.tile_pool(name="dr", bufs=3, space="DRAM"))

        # ---------------- resident weights ----------------
        keys_sb = wpool.tile([128, 8 * H], bf16, tag="keys")
        wa_sb = wpool.tile([128, 8 * H], bf16, tag="wa")
        wab_sb = wpool.tile([1, H], bf16, tag="wab")
        wih_sb = wpool.tile([128, 16 * 384], bf16, tag="wih")
        wihb_sb = wpool.tile([1, 384], bf16, tag="wihb")
        whh_sb = wpool.tile([128, 8 * 384], bf16, tag="whh")
        whhb_sb = wpool.tile([1, 384], bf16, tag="whhb")
        wout_sb = wpool.tile([128, 8 * V], f32, tag="wout")
        woutb_sb = wpool.tile([1, V], f32, tag="woutb")
        wl_sb = wpool.tile([V + 1, H], bf16, tag="wl")
        va_sb = wpool.tile([128, 8], bf16, tag="va")
        id8f_sb = wpool.tile([8, 8], f32, tag="id8f")
        id8b_sb = wpool.tile([8, 8], bf16, tag="id8b")
        id1f_sb = wpool.tile([1, 1], f32, tag="id1f")
        eos_sb = wpool.tile([1, V], f32, tag="eos")
        bv8_sb = wpool.tile([8, 1], f32, tag="bv8")
        bv1_sb = wpool.tile([1, 1], f32, tag="bv1")
        ones8b_sb = wpool.tile([8, 1], bf16, tag="ones8b")
        ones128_sb = wpool.tile([1, 128], f32, tag="ones128")
        one_bf = wpool.tile([1, 1], bf16, tag="onebf")
        one_f = wpool.tile([1, 1], f32, tag="onef")
        kut_sb = wpool.tile([128, 8 * SS], bf16, tag="kut")

        for r in range(8):
            nc.sync.dma_start(
                keys_sb[:, r * H : (r + 1) * H], d_keys.ap()[r * 128 : (r + 1) * 128, :]
            )
            nc.sync.dma_start(
                wa_sb[:, r * H : (r + 1) * H], d_wa.ap()[r * 128 : (r + 1) * 128, :]
            )
            nc.sync.dma_start(
                whh_sb[:, r * 384 : (r + 1) * 384],
                d_whh.ap()[r * 128 : (r + 1) * 128, :],
            )
            nc.sync.dma_start(
                wout_sb[:, r * V : (r + 1) * V], d_wout.ap()[r * 128 : (r + 1) * 128, :]
            )
        for c in range(16):
            nc.sync.dma_start(
                wih_sb[:, c * 384 : (c + 1) * 384],
                d_wih.ap()[c * 128 : (c + 1) * 128, :],
            )
        nc.sync.dma_start(wab_sb[:], d_wa.ap()[H : H + 1, :])
        nc.sync.dma_start(wihb_sb[:], d_wih.ap()[2 * H : 2 * H + 1, :])
        nc.sync.dma_start(whhb_sb[:], d_whh.ap()[H : H + 1, :])
        nc.sync.dma_start(woutb_sb[:], d_wout.ap()[H : H + 1, :])
        nc.sync.dma_start(wl_sb[:], d_wl.ap()[:, :])
        nc.sync.dma_start(va_sb[:], d_va.ap()[:, :])
        nc.sync.dma_start(id8f_sb[:], d_id8f.ap()[:, :])
        nc.sync.dma_start(id8b_sb[:], d_id8b.ap()[:, :])
        nc.sync.dma_start(id1f_sb[:], d_id1f.ap()[:, :])
        nc.sync.dma_start(eos_sb[:], d_eos.ap()[:, :])
        nc.sync.dma_start(bv8_sb[:], d_bv8.ap()[:, :])
        nc.sync.dma_start(bv1_sb[:], d_bv1.ap()[:, :])
        nc.vector.memset(ones8b_sb[:], 1.0)
        nc.vector.memset(ones128_sb[:], 1.0)
        nc.vector.memset(one_bf[:], 1.0)
        nc.vector.memset(one_f[:], 1.0)

        # ---------------- kUT precompute ----------------
        with tc.tile_pool(name="pre", bufs=1) as pre:
            kt_sb = pre.tile([128, 9 * SS], bf16, tag="kt")
            ua_sb = pre.tile([128, 9 * H], bf16, tag="uat")
            for c in range(9):
                rows = 128 if c < 8 else 1
                nc.sync.dma_start(
                    kt_sb[0:rows, c * SS : c * SS + SS],
                    d_keysT.ap()[c * 128 : c * 128 + rows, :],
                )
                nc.sync.dma_start(
                    ua_sb[0:rows, c * H : c * H + H],
                    d_ua.ap()[c * 128 : c * 128 + rows, :],
                )
            for ht in range(8):
                for nb in range(2):
                    pm = ps8.tile([128, 512], mybir.dt.float32, tag="t8")
                    for c in range(9):
                        rows = 128 if c < 8 else 1
                        nc.tensor.matmul(
                            pm[:],
                            ua_sb[0:rows, c * H + ht * 128 : c * H + (ht + 1) * 128],
                            kt_sb[0:rows, c * SS + nb * 512 : c * SS + nb * 512 + 512],
                            start=(c == 0),
                            stop=(c == 8),
                        )
                    nc.scalar.activation(
                        kut_sb[:, ht * SS + nb * 512 : ht * SS + nb * 512 + 512],
                        pm[:],
                        AF.Copy,
                    )

        # ---------------- persistent state ----------------
        h_free = state.tile([8, 128], f32, tag="hfree")
        h_par_bf = state.tile([128, 8], bf16, tag="hparb")
        h_par_f = state.tile([128, 8], f32, tag="hparf")
        h_my = state.tile([1, 128], f32, tag="hmy")
        inp_par = state.tile([V + 1, 1], bf16, tag="inppar")
        inp_f6 = state.tile([1, V + 1], f32, tag="inpf")
        done1 = state.tile([1, 1], f32, tag="done1")

        nc.sync.dma_start(h_free[:], d_h0.ap()[:, :])
        nc.sync.dma_start(h_my[:], d_hmy0.ap()[:, :])
        hi_ps = ps8.tile([128, 8], f32, tag="t8")
        nc.tensor.transpose(hi_ps[:], h_free[:], id8f_sb[:])
        nc.vector.tensor_copy(h_par_bf[:], hi_ps[:])
        nc.vector.tensor_copy(h_par_f[:], hi_ps[:])
        nc.vector.memset(inp_f6[0:1, 0:V], 0.0)
        nc.vector.memset(inp_f6[0:1, V : V + 1], 1.0)
        ip0_ps = psv.tile([V + 1, 1], f32, tag="vec")
        nc.tensor.transpose(ip0_ps[:], inp_f6[:], id1f_sb[:])
        nc.vector.tensor_copy(inp_par[:], ip0_ps[:])
        nc.vector.memset(done1[:], 0.0)

        # ---------------- decode loop ----------------
        for t in range(T):
            # emb in par layout (128, 8) += bias via aug row of Wl
            emb_ps = ps8.tile([128, 8], f32, tag="t8")
            for ht in range(8):
                nc.tensor.matmul(
                    emb_ps[:, ht : ht + 1],
                    wl_sb[:, ht * 128 : (ht + 1) * 128],
                    inp_par[:, 0:1],
                    start=True,
                    stop=True,
                )
            emb_bf = work.tile([128, 8], bf16, tag="embbf")
            nc.vector.tensor_copy(emb_bf[:], emb_ps[:])

            # qW = h @ Wa + ba  -> free layout (1,1024) in two psum halves
            qw_ps = []
            for nb in range(2):
                pm = psv.tile([1, 512], f32, tag="vec")
                for c in range(8):
                    nc.tensor.matmul(
                        pm[:],
                        h_par_bf[:, c : c + 1],
                        wa_sb[:, c * H + nb * 512 : c * H + nb * 512 + 512],
                        start=(c == 0),
                        stop=False,
                    )
                nc.tensor.matmul(
                    pm[:],
                    one_bf[:],
                    wab_sb[:, nb * 512 : nb * 512 + 512],
                    start=False,
                    stop=True,
                )
                qw_ps.append(pm)
            qwf = work.tile([1, H], f32, tag="qwf")
            nc.vector.tensor_copy(qwf[0:1, 0:512], qw_ps[0][:])
            nc.vector.tensor_copy(qwf[0:1, 512:1024], qw_ps[1][:])
            qw8 = work.tile([8, 128], f32, tag="qw8")
            nc.sync.dma_start(qw8[:], qwf[:])
            qwt_ps = ps8.tile([128, 8], f32, tag="t8")
            nc.tensor.transpose(qwt_ps[:], qw8[:], id8f_sb[:])
            qw_par = work.tile([128, 8], f32, tag="qwpar")
            nc.vector.tensor_copy(qw_par[:], qwt_ps[:])

            # tanh(qW + kU) on ScalarE, per h-tile, bias = qW column
            th = big.tile([128, 8 * SS], bf16, tag="th")
            for ht in range(8):
                nc.scalar.activation(
                    th[:, ht * SS : (ht + 1) * SS],
                    kut_sb[:, ht * SS : (ht + 1) * SS],
                    AF.Tanh,
                    bias=qw_par[:, ht : ht + 1],
                )

            # e = Va . tanh  (accumulate over h tiles)
            e_ps = []
            for nb in range(2):
                pm = psv.tile([1, 512], f32, tag="vec")
                for ht in range(8):
                    nc.tensor.matmul(
                        pm[:],
                        va_sb[:, ht : ht + 1],
                        th[:, ht * SS + nb * 512 : ht * SS + nb * 512 + 512],
                        start=(ht == 0),
                        stop=(ht == 7),
                    )
                e_ps.append(pm)
            # w~ = exp(e + bv) straight from PSUM; accum_out gives Z halves
            wf = work.tile([1, H], bf16, tag="wf")
            za = work.tile([1, 2], f32, tag="za")
            nc.scalar.activation(
                wf[0:1, 0:512], e_ps[0][:], AF.Exp,
                bias=bv1_sb[:], accum_out=za[0:1, 0:1],
            )
            nc.scalar.activation(
                wf[0:1, 512:1024], e_ps[1][:], AF.Exp,
                bias=bv1_sb[:], accum_out=za[0:1, 1:2],
            )
            nc.gpsimd.dma_start(d_attns.ap()[t], wf[:])
            w8 = work.tile([8, 128], bf16, tag="w8")
            nc.sync.dma_start(w8[:], wf[:])

            wt_ps = ps8.tile([128, 8], bf16, tag="t8")
            nc.tensor.transpose(wt_ps[:], w8[:], id8b_sb[:])
            w_par = work.tile([128, 8], bf16, tag="wpar")
            nc.vector.tensor_copy(w_par[:], wt_ps[:])

            # ctx~ = w~ @ keys  (unnormalized)
            c_ps = []
            for nb in range(2):
                pm = psv.tile([1, 512], f32, tag="vec")
                for r in range(8):
                    nc.tensor.matmul(
                        pm[:],
                        w_par[:, r : r + 1],
                        keys_sb[:, r * H + nb * 512 : r * H + nb * 512 + 512],
                        start=(r == 0),
                        stop=(r == 7),
                    )
                c_ps.append(pm)

            # AllReduce [ctx~ ; Z]
            arin = dram.tile([1, H + 1], f32, tag="arin")
            arout = dram.tile([1, H + 1], f32, tag="arout")
            cf = work.tile([1, H + 1], f32, tag="cf")
            nc.vector.tensor_copy(cf[0:1, 0:512], c_ps[0][:])
            nc.vector.tensor_copy(cf[0:1, 512:1024], c_ps[1][:])
            nc.vector.reduce_sum(cf[0:1, 1024 : H + 1], za[:], axis=AX.X)
            nc.sync.dma_start(arin[0:1, 0 : H + 1], cf[:])
            if sim:
                nc.sync.dma_start(arout[:], arin[:])
            else:
                nc.gpsimd.collective_compute(
                    "AllReduce",
                    mybir.AluOpType.add,
                    replica_groups=RG,
                    ins=[arin.opt()],
                    outs=[arout.opt()],
                )

            # overlap with AR: gh = W_hh @ h + b_hh ; gi_emb = W_ih[:, :H] @ emb + b_ih
            gh_ps = psv.tile([1, 384], f32, tag="vec")
            for c in range(8):
                nc.tensor.matmul(
                    gh_ps[:],
                    h_par_bf[:, c : c + 1],
                    whh_sb[:, c * 384 : (c + 1) * 384],
                    start=(c == 0),
                    stop=False,
                )
            nc.tensor.matmul(gh_ps[:], one_bf[:], whhb_sb[:], start=False, stop=True)
            gie_ps = psv.tile([1, 384], f32, tag="vec")
            for c in range(8):
                nc.tensor.matmul(
                    gie_ps[:],
                    emb_bf[:, c : c + 1],
                    wih_sb[:, c * 384 : (c + 1) * 384],
                    start=(c == 0),
                    stop=False,
                )
            nc.tensor.matmul(gie_ps[:], one_bf[:], wihb_sb[:], start=False, stop=True)

            # post-AR: ctx, 1/Z
            ctx8 = work.tile([8, 128], f32, tag="ctx8")
            nc.sync.dma_start(ctx8[:], arout[0:1, 0:1024])
            zg = work.tile([1, 1], f32, tag="zg")
            nc.sync.dma_start(zg[:], arout[0:1, 1024 : H + 1])
            nc.gpsimd.dma_start(d_zout.ap()[t], arout[0:1, 1024 : H + 1])
            prec = work.tile([1, 1], f32, tag="prec")
            nc.vector.reciprocal(prec[:], zg[:])
            ctxt_ps = ps8.tile([128, 8], f32, tag="t8")
            nc.tensor.transpose(ctxt_ps[:], ctx8[:], id8f_sb[:])
            ctx_par = work.tile([128, 8], bf16, tag="ctxpar")
            nc.vector.tensor_copy(ctx_par[:], ctxt_ps[:])

            gic_ps = psv.tile([1, 384], f32, tag="vec")
            for c in range(8):
                nc.tensor.matmul(
                    gic_ps[:],
                    ctx_par[:, c : c + 1],
                    wih_sb[:, (8 + c) * 384 : (9 + c) * 384],
                    start=(c == 0),
                    stop=(c == 7),
                )

            # gates (free layout, partition 0)
            gi = work.tile([1, 384], f32, tag="gi")
            nc.vector.tensor_scalar_mul(gi[:], gic_ps[:], prec[:])
            nc.vector.tensor_add(gi[:], gi[:], gie_ps[:])
            rz_in = work.tile([1, 256], f32, tag="rzin")
            nc.vector.tensor_add(rz_in[:], gi[0:1, 0:256], gh_ps[0:1, 0:256])
            rz = work.tile([1, 256], f32, tag="rz")
            nc.scalar.activation(rz[:], rz_in[:], AF.Tanh, scale=0.5)
            nc.vector.tensor_scalar(rz[:], rz[:], 0.5, 0.5, ALU.mult, ALU.add)
            nin = work.tile([1, 128], f32, tag="nin")
            nc.vector.tensor_mul(nin[:], rz[0:1, 0:128], gh_ps[0:1, 256:384])
            nc.vector.tensor_add(nin[:], nin[:], gi[0:1, 256:384])
            nn = work.tile([1, 128], f32, tag="nn")
            nc.scalar.activation(nn[:], nin[:], AF.Tanh)
            d1 = work.tile([1, 128], f32, tag="d1")
            nc.vector.tensor_sub(d1[:], h_my[:], nn[:])
            nc.vector.tensor_mul(d1[:], d1[:], rz[0:1, 128:256])
            hn = work.tile([1, 128], f32, tag="hn")
            nc.vector.tensor_add(hn[:], nn[:], d1[:])
            # freeze h after done (uses done from PREVIOUS step)
            d2 = work.tile([1, 128], f32, tag="d2")
            nc.vector.tensor_sub(d2[:], h_my[:], hn[:])
            nc.vector.scalar_tensor_tensor(
                h_my[:], d2[:], done1[:], hn[:], ALU.mult, ALU.add
            )

            # AllGather h
            agin = dram.tile([1, 128], f32, tag="agin")
            agout = dram.tile([8, 128], f32, tag="agout")
            nc.sync.dma_start(agin[:], h_my[:])
            if sim:
                nc.sync.dma_start(agout[0:1, :], agin[:])
            else:
                nc.gpsimd.collective_compute(
                    "AllGather",
                    mybir.AluOpType.bypass,
                    replica_groups=RG,
                    ins=[agin.opt()],
                    outs=[agout.opt()],
                )
            nc.sync.dma_start(h_free[:], agout[:])
            ht_ps = ps8.tile([128, 8], f32, tag="t8")
            nc.tensor.transpose(ht_ps[:], h_free[:], id8f_sb[:])
            nc.vector.tensor_copy(h_par_bf[:], ht_ps[:])
            nc.vector.tensor_copy(h_par_f[:], ht_ps[:])

            # logits (f32 path to keep the EOS decision faithful)
            lg_ps = psv.tile([1, V], f32, tag="vec")
            for c in range(8):
                nc.tensor.matmul(
                    lg_ps[:],
                    h_par_f[:, c : c + 1],
                    wout_sb[:, c * V : (c + 1) * V],
                    start=(c == 0),
                    stop=False,
                )
            nc.tensor.matmul(lg_ps[:], one_f[:], woutb_sb[:], start=False, stop=True)
            pexp = work.tile([1, V], f32, tag="pexp")
            nc.scalar.activation(pexp[:], lg_ps[:], AF.Exp)
            psm = work.tile([1, 1], f32, tag="psm")
            nc.vector.reduce_sum(psm[:], pexp[:], axis=AX.X)
            pr = work.tile([1, 1], f32, tag="pr")
            nc.vector.reciprocal(pr[:], psm[:])
            probs = work.tile([1, V], f32, tag="probs")
            nc.vector.tensor_scalar_mul(probs[:], pexp[:], pr[:])
            if t >= 5:
                pmax = work.tile([1, 1], f32, tag="pmax")
                nc.vector.reduce_max(pmax[:], pexp[:], axis=AX.X)
                trig = work.tile([1, 1], f32, tag="trig")
                nc.vector.tensor_tensor(
                    trig[:], pexp[0:1, V - 1 : V], pmax[:], op=ALU.is_ge
                )
                nc.vector.tensor_max(done1[:], done1[:], trig[:])

            # out_t = probs + done*(EOS - probs) ; inp = probs + done*(inp - probs)
            o1 = work.tile([1, V], f32, tag="o1")
            nc.vector.tensor_sub(o1[:], eos_sb[:], probs[:])
            nc.vector.scalar_tensor_tensor(
                o1[:], o1[:], done1[:], probs[:], ALU.mult, ALU.add
            )
            nc.gpsimd.dma_start(d_outs.ap()[t], o1[:])
            i1 = work.tile([1, V], f32, tag="i1")
            nc.vector.tensor_sub(i1[:], inp_f6[0:1, 0:V], probs[:])
            nc.vector.scalar_tensor_tensor(
                inp_f6[0:1, 0:V], i1[:], done1[:], probs[:], ALU.mult, ALU.add
            )
            ip_ps = psv.tile([V + 1, 1], f32, tag="vec")
            nc.tensor.transpose(ip_ps[:], inp_f6[:], id1f_sb[:])
            nc.vector.tensor_copy(inp_par[:], ip_ps[:])

        nc.sync.dma_start(d_hfin.ap()[:, :], h_free[:])

    nc.compile()
    return nc


def _get_nc():
    if "nc" not in _CACHE:
        _CACHE["nc"] = _build()
    return _CACHE["nc"]


def kernel(**inputs):
    from concourse.bass_utils import run_bass_kernel_spmd

    f32 = np.float32
    keys = np.asarray(inputs["encoder_outputs"], f32)[0]  # (S, H)
    h0 = np.asarray(inputs["encoder_hidden"], f32)[0, 0]  # (H,)
    Wa = np.asarray(inputs["Wa"], f32)
    ba = np.asarray(inputs["ba"], f32)
    Ua = np.asarray(inputs["Ua"], f32)
    bu = np.asarray(inputs["bu"], f32)
    Va = np.asarray(inputs["Va"], f32)
    bv = np.asarray(inputs["bv"], f32)
    Wl = np.asarray(inputs["Wl"], f32)
    bl = np.asarray(inputs["bl"], f32)
    W_ih = np.asarray(inputs["W_ih"], f32)
    b_ih = np.asarray(inputs["b_ih"], f32)
    W_hh = np.asarray(inputs["W_hh"], f32)
    b_hh = np.asarray(inputs["b_hh"], f32)
    W_out = np.asarray(inputs["W_out"], f32)
    b_out = np.asarray(inputs["b_out"], f32)

    Ua_aug = np.vstack([Ua, bu[None, :]]).astype(BF)
    Wa_aug = np.vstack([Wa, ba[None, :]]).astype(BF)
    Wl_aug = np.vstack([Wl, bl[None, :]]).astype(BF)
    Wout_aug = np.vstack([W_out, b_out[None, :]]).astype(f32)
    va_par = np.ascontiguousarray(Va[:, 0].reshape(8, 128).T).astype(BF)

    shared = {
        "ua": Ua_aug,
        "wa": Wa_aug,
        "wl": Wl_aug,
        "wout": Wout_aug,
        "va": va_par,
        "h0": np.ascontiguousarray(h0.reshape(8, 128)),
        "id8f": np.eye(8, dtype=f32),
        "id8b": np.eye(8).astype(BF),
        "id1f": np.ones((1, 1), f32),
        "eos": np.array([[0, 0, 0, 0, 1]], f32),
        "bv8": np.full((8, 1), float(bv[0]), f32),
        "bv1": np.full((1, 1), float(bv[0]), f32),
    }
    in_maps = []
    for k in range(NCORES):
        sl = slice(k * SS, (k + 1) * SS)
        keys_k = keys[sl]
        keysT_aug = np.vstack([keys_k.T, np.ones((1, SS), f32)]).astype(BF)
        rows = np.r_[
            k * 128 : (k + 1) * 128,
            H + k * 128 : H + (k + 1) * 128,
            2 * H + k * 128 : 2 * H + (k + 1) * 128,
        ]
        WihT_aug = np.vstack([W_ih[rows, :].T, b_ih[rows][None, :]]).astype(BF)
        WhhT_aug = np.vstack([W_hh[rows, :].T, b_hh[rows][None, :]]).astype(BF)
        m = dict(shared)
        m.update(
            {
                "keys": keys_k.astype(BF),
                "keysT": keysT_aug,
                "wih": WihT_aug,
                "whh": WhhT_aug,
                "hmy0": np.ascontiguousarray(h0[k * 128 : (k + 1) * 128][None, :]),
            }
        )
        in_maps.append(m)

    nc = _get_nc()
    res = run_bass_kernel_spmd(nc, in_maps, core_ids=list(range(NCORES)))
    r = res.results
    outs = np.asarray(r[0]["outs"], f32).reshape(T, V)
    hfin = np.asarray(r[0]["hfin"], f32).reshape(H)
    Z = np.asarray(r[0]["zout"], f32).reshape(T)
    att = np.concatenate(
        [np.asarray(r[k]["attns"]).astype(f32).reshape(T, SS) for k in range(NCORES)],
        axis=1,
    )
    attns = att / Z[:, None]
    return outs[None], hfin[None, None], attns[None]


if __name__ == "__main__":
    _get_nc()
    print("build+compile OK")


# revision 15
# speedup vs baseline: 1.0490x; 1.0459x over previous
"""AttnDecoderRNN on 8 trn2 NeuronCores.

Strategy: shard encoder S=8192 across 8 cores (1024 each). All loop-invariant
weights live SBUF-resident in bf16. Per decode step:
  - qW = h@Wa (TensorE, vec-as-weights trick), tanh(qW+kU) on ScalarE with
    per-partition bias, e = Va-reduction (TensorE), exp (ScalarE),
    ctx~ = w~ @ keys (TensorE)  -- all on the local S-shard.
  - ONE AllReduce of [ctx~ (1024); Z_part] ; GRU gh/gi_emb matmuls overlap it.
  - GRU gates + EOS logic computed redundantly on every core (tiny).
  - ONE AllGather of the core's h-shard (128) -> full h everywhere.
Attention weights are stored unnormalized (bf16) + Z per step; the host
divides during unshard.
"""

import sys
import numpy as np

sys.path.insert(0, "/opt/trn_rl_repo")

import ml_dtypes

H = 1024
S = 8192
V = 5
T = 23
NCORES = 8
SS = S // NCORES  # 1024
BF = ml_dtypes.bfloat16

_CACHE = {}


def _build(sim=False):
    from concourse import bass, bacc, tile, mybir
    from contextlib import ExitStack

    f32 = mybir.dt.float32
    bf16 = mybir.dt.bfloat16
    AF = mybir.ActivationFunctionType
    ALU = mybir.AluOpType
    AX = mybir.AxisListType

    nc = bacc.Bacc(
        "TRN2",
        target_bir_lowering=False,
        debug=False,
        enable_asserts=True,
        num_devices=1 if sim else NCORES,
    )

    # ---- per-core DRAM inputs -------------------------------------------
    d_keys = nc.dram_tensor("keys", [SS, H], bf16, kind="ExternalInput")
    d_keysT = nc.dram_tensor("keysT", [H + 1, SS], bf16, kind="ExternalInput")
    d_ua = nc.dram_tensor("ua", [H + 1, H], bf16, kind="ExternalInput")
    d_wa = nc.dram_tensor("wa", [H + 1, H], bf16, kind="ExternalInput")
    d_wl = nc.dram_tensor("wl", [V + 1, H], bf16, kind="ExternalInput")
    d_wih = nc.dram_tensor("wih", [2 * H + 1, 384], bf16, kind="ExternalInput")
    d_whh = nc.dram_tensor("whh", [H + 1, 384], bf16, kind="ExternalInput")
    d_wout = nc.dram_tensor("wout", [H + 1, V], f32, kind="ExternalInput")
    d_va = nc.dram_tensor("va", [128, 8], bf16, kind="ExternalInput")
    d_h0 = nc.dram_tensor("h0", [8, 128], f32, kind="ExternalInput")
    d_hmy0 = nc.dram_tensor("hmy0", [1, 128], f32, kind="ExternalInput")
    d_id8f = nc.dram_tensor("id8f", [8, 8], f32, kind="ExternalInput")
    d_id8b = nc.dram_tensor("id8b", [8, 8], bf16, kind="ExternalInput")
    d_id1f = nc.dram_tensor("id1f", [1, 1], f32, kind="ExternalInput")
    d_eos = nc.dram_tensor("eos", [1, V], f32, kind="ExternalInput")
    d_bv8 = nc.dram_tensor("bv8", [8, 1], f32, kind="ExternalInput")
    d_bv1 = nc.dram_tensor("bv1", [1, 1], f32, kind="ExternalInput")

    # ---- per-core DRAM outputs ------------------------------------------
    d_outs = nc.dram_tensor("outs", [T, V], f32, kind="ExternalOutput")
    d_hfin = nc.dram_tensor("hfin", [8, 128], f32, kind="ExternalOutput")
    d_attns = nc.dram_tensor("attns", [T, 8, 128], bf16, kind="ExternalOutput")
    d_zout = nc.dram_tensor("zout", [T, 1], f32, kind="ExternalOutput")

    RG = [list(range(NCORES))]

    with tile.TileContext(nc) as tc, ExitStack() as ctx:
        wpool = ctx.enter_context(tc.tile_pool(name="w", bufs=1))
        state = ctx.enter_context(tc.tile_pool(name="st", bufs=1))
        work = ctx.enter_context(tc.tile_pool(name="wk", bufs=3))
        big = ctx.enter_context(tc.tile_pool(name="big", bufs=1))
        ps8 = ctx.enter_context(tc.tile_pool(name="p8", bufs=2, space="PSUM"))
        psv = ctx.enter_context(tc.tile_pool(name="pv", bufs=4, space="PSUM"))
        dram = ctx.enter_context(tc.tile_pool(name="dr", bufs=3, space="DRAM"))

        # ---------------- resident weights ----------------
        keys_sb = wpool.tile([128, 8 * H], bf16, tag="keys")
        wa_sb = wpool.tile([128, 8 * H], bf16, tag="wa")
        wab_sb = wpool.tile([1, H], bf16, tag="wab")
        wih_sb = wpool.tile([128, 16 * 384], bf16, tag="wih")
        wihb_sb = wpool.tile([1, 384], bf16, tag="wihb")
        whh_sb = wpool.tile([128, 8 * 384], bf16, tag="whh")
        whhb_sb = wpool.tile([1, 384], bf16, tag="whhb")
        wout_sb = wpool.tile([128, 8 * V], f32, tag="wout")
        woutb_sb = wpool.tile([1, V], f32, tag="woutb")
        wl_sb = wpool.tile([V + 1, H], bf16, tag="wl")
        va_sb = wpool.tile([128, 8], bf16, tag="va")
        id8f_sb = wpool.tile([8, 8], f32, tag="id8f")
        id8b_sb = wpool.tile([8, 8], bf16, tag="id8b")
        id1f_sb = wpool.tile([1, 1], f32, tag="id1f")
        eos_sb = wpool.tile([1, V], f32, tag="eos")
        bv8_sb = wpool.tile([8, 1], f32, tag="bv8")
        bv1_sb = wpool.tile([1, 1], f32, tag="bv1")
        ones8b_sb = wpool.tile([8, 1], bf16, tag="ones8b")
        ones128_sb = wpool.tile([1, 128], f32, tag="ones128")
        one_bf = wpool.tile([1, 1], bf16, tag="onebf")
        one_f = wpool.tile([1, 1], f32, tag="onef")
        kut_sb = wpool.tile([128, 8 * SS], bf16, tag="kut")

        for r in range(8):
            nc.sync.dma_start(
                keys_sb[:, r * H : (r + 1) * H], d_keys.ap()[r * 128 : (r + 1) * 128, :]
            )
            nc.sync.dma_start(
                wa_sb[:, r * H : (r + 1) * H], d_wa.ap()[r * 128 : (r + 1) * 128, :]
            )
            nc.sync.dma_start(
                whh_sb[:, r * 384 : (r + 1) * 384],
                d_whh.ap()[r * 128 : (r + 1) * 128, :],
            )
            nc.sync.dma_start(
                wout_sb[:, r * V : (r + 1) * V], d_wout.ap()[r * 128 : (r + 1) * 128, :]
            )
        for c in range(16):
            nc.sync.dma_start(
                wih_sb[:, c * 384 : (c + 1) * 384],
                d_wih.ap()[c * 128 : (c + 1) * 128, :],
            )
        nc.sync.dma_start(wab_sb[:], d_wa.ap()[H : H + 1, :])
        nc.sync.dma_start(wihb_sb[:], d_wih.ap()[2 * H : 2 * H + 1, :])
        nc.sync.dma_start(whhb_sb[:], d_whh.ap()[H : H + 1, :])
        nc.sync.dma_start(woutb_sb[:], d_wout.ap()[H : H + 1, :])
        nc.sync.dma_start(wl_sb[:], d_wl.ap()[:, :])
        nc.sync.dma_start(va_sb[:], d_va.ap()[:, :])
        nc.sync.dma_start(id8f_sb[:], d_id8f.ap()[:, :])
        nc.sync.dma_start(id8b_sb[:], d_id8b.ap()[:, :])
        nc.sync.dma_start(id1f_sb[:], d_id1f.ap()[:, :])
        nc.sync.dma_start(eos_sb[:], d_eos.ap()[:, :])
        nc.sync.dma_start(bv8_sb[:], d_bv8.ap()[:, :])
        nc.sync.dma_start(bv1_sb[:], d_bv1.ap()[:, :])
        nc.vector.memset(ones8b_sb[:], 1.0)
        nc.vector.memset(ones128_sb[:], 1.0)
        nc.vector.memset(one_bf[:], 1.0)
        nc.vector.memset(one_f[:], 1.0)

        # ---------------- kUT precompute ----------------
        with tc.tile_pool(name="pre", bufs=1) as pre:
            kt_sb = pre.tile([128, 9 * SS], bf16, tag="kt")
            ua_sb = pre.tile([128, 9 * H], bf16, tag="uat")
            for c in range(9):
                rows = 128 if c < 8 else 1
                nc.sync.dma_start(
                    kt_sb[0:rows, c * SS : c * SS + SS],
                    d_keysT.ap()[c * 128 : c * 128 + rows, :],
                )
                nc.sync.dma_start(
                    ua_sb[0:rows, c * H : c * H + H],
                    d_ua.ap()[c * 128 : c * 128 + rows, :],
                )
            for ht in range(8):
                for nb in range(2):
                    pm = ps8.tile([128, 512], mybir.dt.float32, tag="t8")
                    for c in range(9):
                        rows = 128 if c < 8 else 1
                        nc.tensor.matmul(
                            pm[:],
                            ua_sb[0:rows, c * H + ht * 128 : c * H + (ht + 1) * 128],
                            kt_sb[0:rows, c * SS + nb * 512 : c * SS + nb * 512 + 512],
                            start=(c == 0),
                            stop=(c == 8),
                        )
                    nc.scalar.activation(
                        kut_sb[:, ht * SS + nb * 512 : ht * SS + nb * 512 + 512],
                        pm[:],
                        AF.Copy,
                    )

        # ---------------- persistent state ----------------
        h_free = state.tile([8, 128], f32, tag="hfree")
        h_par_bf = state.tile([128, 8], bf16, tag="hparb")
        h_par_f = state.tile([128, 8], f32, tag="hparf")
        h_my = state.tile([1, 128], f32, tag="hmy")
        inp_par = state.tile([V + 1, 1], bf16, tag="inppar")
        inp_f6 = state.tile([1, V + 1], f32, tag="inpf")
        done1 = state.tile([1, 1], f32, tag="done1")

        nc.sync.dma_start(h_free[:], d_h0.ap()[:, :])
        nc.sync.dma_start(h_my[:], d_hmy0.ap()[:, :])
        hi_ps = ps8.tile([128, 8], f32, tag="t8")
        nc.tensor.transpose(hi_ps[:], h_free[:], id8f_sb[:])
        nc.vector.tensor_copy(h_par_bf[:], hi_ps[:])
        nc.vector.tensor_copy(h_par_f[:], hi_ps[:])
        nc.vector.memset(inp_f6[0:1, 0:V], 0.0)
        nc.vector.memset(inp_f6[0:1, V : V + 1], 1.0)
        ip0_ps = psv.tile([V + 1, 1], f32, tag="vec")
        nc.tensor.transpose(ip0_ps[:], inp_f6[:], id1f_sb[:])
        nc.vector.tensor_copy(inp_par[:], ip0_ps[:])
        nc.vector.memset(done1[:], 0.0)

        # ---------------- decode loop ----------------
        for t in range(T):
            # emb in par layout (128, 8) += bias via aug row of Wl
            emb_ps = ps8.tile([128, 8], f32, tag="t8")
            for ht in range(8):
                nc.tensor.matmul(
                    emb_ps[:, ht : ht + 1],
                    wl_sb[:, ht * 128 : (ht + 1) * 128],
                    inp_par[:, 0:1],
                    start=True,
                    stop=True,
                )
            emb_bf = work.tile([128, 8], bf16, tag="embbf")
            nc.vector.tensor_copy(emb_bf[:], emb_ps[:])

            # qW = h @ Wa + ba  -> free layout (1,1024) in two psum halves
            qw_ps = []
            for nb in range(2):
                pm = psv.tile([1, 512], f32, tag="vec")
                for c in range(8):
                    nc.tensor.matmul(
                        pm[:],
                        h_par_bf[:, c : c + 1],
                        wa_sb[:, c * H + nb * 512 : c * H + nb * 512 + 512],
                        start=(c == 0),
                        stop=False,
                    )
                nc.tensor.matmul(
                    pm[:],
                    one_bf[:],
                    wab_sb[:, nb * 512 : nb * 512 + 512],
                    start=False,
                    stop=True,
                )
                qw_ps.append(pm)
            qwf = work.tile([1, H], f32, tag="qwf")
            qw8a = work.tile([4, 128], f32, tag="qw8a")
            qw8b = work.tile([4, 128], f32, tag="qw8b")
            qwt_ps = ps8.tile([128, 8], f32, tag="t8")
            qw_par = work.tile([128, 8], f32, tag="qwpar")
            for hh, t8 in ((0, qw8a), (1, qw8b)):
                nc.vector.tensor_copy(
                    qwf[0:1, hh * 512 : hh * 512 + 512], qw_ps[hh][:]
                )
                nc.sync.dma_start(t8[:], qwf[0:1, hh * 512 : hh * 512 + 512])
                nc.tensor.transpose(
                    qwt_ps[:, hh * 4 : hh * 4 + 4], t8[:], id8f_sb[0:4, 0:4]
                )
                nc.vector.tensor_copy(
                    qw_par[:, hh * 4 : hh * 4 + 4], qwt_ps[:, hh * 4 : hh * 4 + 4]
                )

            # tanh(qW + kU) on ScalarE, per h-tile, bias = qW column
            th = big.tile([128, 8 * SS], bf16, tag="th")
            for ht in range(8):
                nc.scalar.activation(
                    th[:, ht * SS : (ht + 1) * SS],
                    kut_sb[:, ht * SS : (ht + 1) * SS],
                    AF.Tanh,
                    bias=qw_par[:, ht : ht + 1],
                )

            # e = Va . tanh  (accumulate over h tiles)
            e_ps = []
            for nb in range(2):
                pm = psv.tile([1, 512], f32, tag="vec")
                for ht in range(8):
                    nc.tensor.matmul(
                        pm[:],
                        va_sb[:, ht : ht + 1],
                        th[:, ht * SS + nb * 512 : ht * SS + nb * 512 + 512],
                        start=(ht == 0),
                        stop=(ht == 7),
                    )
                e_ps.append(pm)
            # w~ = exp(e + bv) straight from PSUM; accum_out gives Z halves
            wf = work.tile([1, H], bf16, tag="wf")
            za = work.tile([1, 2], f32, tag="za")
            nc.scalar.activation(
                wf[0:1, 0:512], e_ps[0][:], AF.Exp,
                bias=bv1_sb[:], accum_out=za[0:1, 0:1],
            )
            nc.scalar.activation(
                wf[0:1, 512:1024], e_ps[1][:], AF.Exp,
                bias=bv1_sb[:], accum_out=za[0:1, 1:2],
            )
            nc.gpsimd.dma_start(d_attns.ap()[t], wf[:])
            w8a = work.tile([4, 128], bf16, tag="w8a")
            w8b = work.tile([4, 128], bf16, tag="w8b")
            wt_ps = ps8.tile([128, 8], bf16, tag="t8")
            w_par = work.tile([128, 8], bf16, tag="wpar")
            for hh, t8 in ((0, w8a), (1, w8b)):
                nc.sync.dma_start(t8[:], wf[0:1, hh * 512 : hh * 512 + 512])
                nc.tensor.transpose(
                    wt_ps[:, hh * 4 : hh * 4 + 4], t8[:], id8b_sb[0:4, 0:4]
                )
                nc.vector.tensor_copy(
                    w_par[:, hh * 4 : hh * 4 + 4], wt_ps[:, hh * 4 : hh * 4 + 4]
                )

            # ctx~ = w~ @ keys  (unnormalized)
            c_ps = []
            for nb in range(2):
                pm = psv.tile([1, 512], f32, tag="vec")
                for r in range(8):
                    nc.tensor.matmul(
                        pm[:],
                        w_par[:, r : r + 1],
                        keys_sb[:, r * H + nb * 512 : r * H + nb * 512 + 512],
                        start=(r == 0),
                        stop=(r == 7),
                    )
                c_ps.append(pm)

            # AllReduce [ctx~ ; Z]
            arin = dram.tile([1, H + 1], f32, tag="arin")
            arout = dram.tile([1, H + 1], f32, tag="arout")
            cf = work.tile([1, H + 1], f32, tag="cf")
            nc.vector.tensor_copy(cf[0:1, 0:512], c_ps[0][:])
            nc.vector.tensor_copy(cf[0:1, 512:1024], c_ps[1][:])
            nc.vector.reduce_sum(cf[0:1, 1024 : H + 1], za[:], axis=AX.X)
            nc.sync.dma_start(arin[0:1, 0 : H + 1], cf[:])
            if sim:
                nc.sync.dma_start(arout[:], arin[:])
            else:
                nc.gpsimd.collective_compute(
                    "AllReduce",
                    mybir.AluOpType.add,
                    replica_groups=RG,
                    ins=[arin.opt()],
                    outs=[arout.opt()],
                )

            # overlap with AR: gh = W_hh @ h + b_hh ; gi_emb = W_ih[:, :H] @ emb + b_ih
            gh_ps = psv.tile([1, 384], f32, tag="vec")
            for c in range(8):
                nc.tensor.matmul(
                    gh_ps[:],
                    h_par_bf[:, c : c + 1],
                    whh_sb[:, c * 384 : (c + 1) * 384],
                    start=(c == 0),
                    stop=False,
                )
            nc.tensor.matmul(gh_ps[:], one_bf[:], whhb_sb[:], start=False, stop=True)
            gie_ps = psv.tile([1, 384], f32, tag="vec")
            for c in range(8):
                nc.tensor.matmul(
                    gie_ps[:],
                    emb_bf[:, c : c + 1],
                    wih_sb[:, c * 384 : (c + 1) * 384],
                    start=(c == 0),
                    stop=False,
                )
            nc.tensor.matmul(gie_ps[:], one_bf[:], wihb_sb[:], start=False, stop=True)

            # post-AR: ctx, 1/Z
            ctx8 = work.tile([8, 128], f32, tag="ctx8")
            nc.sync.dma_start(ctx8[:], arout[0:1, 0:1024])
            zg = work.tile([1, 1], f32, tag="zg")
            nc.sync.dma_start(zg[:], arout[0:1, 1024 : H + 1])
            nc.gpsimd.dma_start(d_zout.ap()[t], arout[0:1, 1024 : H + 1])
            prec = work.tile([1, 1], f32, tag="prec")
            nc.vector.reciprocal(prec[:], zg[:])
            ctxt_ps = ps8.tile([128, 8], f32, tag="t8")
            nc.tensor.transpose(ctxt_ps[:], ctx8[:], id8f_sb[:])
            ctx_par = work.tile([128, 8], bf16, tag="ctxpar")
            nc.vector.tensor_copy(ctx_par[:], ctxt_ps[:])

            gic_ps = psv.tile([1, 384], f32, tag="vec")
            for c in range(8):
                nc.tensor.matmul(
                    gic_ps[:],
                    ctx_par[:, c : c + 1],
                    wih_sb[:, (8 + c) * 384 : (9 + c) * 384],
                    start=(c == 0),
                    stop=(c == 7),
                )

            # gates (free layout, partition 0)
            gi = work.tile([1, 384], f32, tag="gi")
            nc.vector.tensor_scalar_mul(gi[:], gic_ps[:], prec[:])
            nc.vector.tensor_add(gi[:], gi[:], gie_ps[:])
            rz_in = work.tile([1, 256], f32, tag="rzin")
            nc.vector.tensor_add(rz_in[:], gi[0:1, 0:256], gh_ps[0:1, 0:256])
            rz = work.tile([1, 256], f32, tag="rz")
            nc.scalar.activation(rz[:], rz_in[:], AF.Tanh, scale=0.5)
            nc.vector.tensor_scalar(rz[:], rz[:], 0.5, 0.5, ALU.mult, ALU.add)
            nin = work.tile([1, 128], f32, tag="nin")
            nc.vector.tensor_mul(nin[:], rz[0:1, 0:128], gh_ps[0:1, 256:384])
            nc.vector.tensor_add(nin[:], nin[:], gi[0:1, 256:384])
            nn = work.tile([1, 128], f32, tag="nn")
            nc.scalar.activation(nn[:], nin[:], AF.Tanh)
            d1 = work.tile([1, 128], f32, tag="d1")
            nc.vector.tensor_sub(d1[:], h_my[:], nn[:])
            nc.vector.tensor_mul(d1[:], d1[:], rz[0:1, 128:256])
            hn = work.tile([1, 128], f32, tag="hn")
            nc.vector.tensor_add(hn[:], nn[:], d1[:])
            # freeze h after done (uses done from PREVIOUS step)
            d2 = work.tile([1, 128], f32, tag="d2")
            nc.vector.tensor_sub(d2[:], h_my[:], hn[:])
            nc.vector.scalar_tensor_tensor(
                h_my[:], d2[:], done1[:], hn[:], ALU.mult, ALU.add
            )

            # AllGather h
            agin = dram.tile([1, 128], f32, tag="agin")
            agout = dram.tile([8, 128], f32, tag="agout")
            nc.sync.dma_start(agin[:], h_my[:])
            if sim:
                nc.sync.dma_start(agout[0:1, :], agin[:])
            else:
                nc.gpsimd.collective_compute(
                    "AllGather",
                    mybir.AluOpType.bypass,
                    replica_groups=RG,
                    ins=[agin.opt()],
                    outs=[agout.opt()],
                )
            nc.sync.dma_start(h_free[:], agout[:])
            ht_ps = ps8.tile([128, 8], f32, tag="t8")
            nc.tensor.transpose(ht_ps[:], h_free[:], id8f_sb[:])
            nc.vector.tensor_copy(h_par_bf[:], ht_ps[:])
            nc.vector.tensor_copy(h_par_f[:], ht_ps[:])

            # logits (f32 path to keep the EOS decision faithful)
            lg_ps = psv.tile([1, V], f32, tag="vec")
            for c in range(8):
                nc.tensor.matmul(
                    lg_ps[:],
                    h_par_f[:, c : c + 1],
                    wout_sb[:, c * V : (c + 1) * V],
                    start=(c == 0),
                    stop=False,
                )
            nc.tensor.matmul(lg_ps[:], one_f[:], woutb_sb[:], start=False, stop=True)
            pexp = work.tile([1, V], f32, tag="pexp")
            nc.scalar.activation(pexp[:], lg_ps[:], AF.Exp)
            psm = work.tile([1, 1], f32, tag="psm")
            nc.vector.reduce_sum(psm[:], pexp[:], axis=AX.X)
            pr = work.tile([1, 1], f32, tag="pr")
            nc.vector.reciprocal(pr[:], psm[:])
            probs = work.tile([1, V], f32, tag="probs")
            nc.vector.tensor_scalar_mul(probs[:], pexp[:], pr[:])
            if t >= 5:
                pmax = work.tile([1, 1], f32, tag="pmax")
                nc.vector.reduce_max(pmax[:], pexp[:], axis=AX.X)
                trig = work.tile([1, 1], f32, tag="trig")
                nc.vector.tensor_tensor(
                    trig[:], pexp[0:1, V - 1 : V], pmax[:], op=ALU.is_ge
                )
                nc.vector.tensor_max(done1[:], done1[:], trig[:])

            # out_t = probs + done*(EOS - probs) ; inp = probs + done*(inp - probs)
            o1 = work.tile([1, V], f32, tag="o1")
            nc.vector.tensor_sub(o1[:], eos_sb[:], probs[:])
            nc.vector.scalar_tensor_tensor(
                o1[:], o1[:], done1[:], probs[:], ALU.mult, ALU.add
            )
            nc.gpsimd.dma_start(d_outs.ap()[t], o1[:])
            i1 = work.tile([1, V], f32, tag="i1")
            nc.vector.tensor_sub(i1[:], inp_f6[0:1, 0:V], probs[:])
            nc.vector.scalar_tensor_tensor(
                inp_f6[0:1, 0:V], i1[:], done1[:], probs[:], ALU.mult, ALU.add
            )
            ip_ps = psv.tile([V + 1, 1], f32, tag="vec")
            nc.tensor.transpose(ip_ps[:], inp_f6[:], id1f_sb[:])
            nc.vector.tensor_copy(inp_par[:], ip_ps[:])

        nc.sync.dma_start(d_hfin.ap()[:, :], h_free[:])

    nc.compile()
    return nc


def _get_nc():
    if "nc" not in _CACHE:
        _CACHE["nc"] = _build()
    return _CACHE["nc"]


def kernel(**inputs):
    from concourse.bass_utils import run_bass_kernel_spmd

    f32 = np.float32
    keys = np.asarray(inputs["encoder_outputs"], f32)[0]  # (S, H)
    h0 = np.asarray(inputs["encoder_hidden"], f32)[0, 0]  # (H,)
    Wa = np.asarray(inputs["Wa"], f32)
    ba = np.asarray(inputs["ba"], f32)
    Ua = np.asarray(inputs["Ua"], f32)
    bu = np.asarray(inputs["bu"], f32)
    Va = np.asarray(inputs["Va"], f32)
    bv = np.asarray(inputs["bv"], f32)
    Wl = np.asarray(inputs["Wl"], f32)
    bl = np.asarray(inputs["bl"], f32)
    W_ih = np.asarray(inputs["W_ih"], f32)
    b_ih = np.asarray(inputs["b_ih"], f32)
    W_hh = np.asarray(inputs["W_hh"], f32)
    b_hh = np.asarray(inputs["b_hh"], f32)
    W_out = np.asarray(inputs["W_out"], f32)
    b_out = np.asarray(inputs["b_out"], f32)

    Ua_aug = np.vstack([Ua, bu[None, :]]).astype(BF)
    Wa_aug = np.vstack([Wa, ba[None, :]]).astype(BF)
    Wl_aug = np.vstack([Wl, bl[None, :]]).astype(BF)
    Wout_aug = np.vstack([W_out, b_out[None, :]]).astype(f32)
    va_par = np.ascontiguousarray(Va[:, 0].reshape(8, 128).T).astype(BF)

    shared = {
        "ua": Ua_aug,
        "wa": Wa_aug,
        "wl": Wl_aug,
        "wout": Wout_aug,
        "va": va_par,
        "h0": np.ascontiguousarray(h0.reshape(8, 128)),
        "id8f": np.eye(8, dtype=f32),
        "id8b": np.eye(8).astype(BF),
        "id1f": np.ones((1, 1), f32),
        "eos": np.array([[0, 0, 0, 0, 1]], f32),
        "bv8": np.full((8, 1), float(bv[0]), f32),
        "bv1": np.full((1, 1), float(bv[0]), f32),
    }
    in_maps = []
    for k in range(NCORES):
        sl = slice(k * SS, (k + 1) * SS)
        keys_k = keys[sl]
        keysT_aug = np.vstack([keys_k.T, np.ones((1, SS), f32)]).astype(BF)
        rows = np.r_[
            k * 128 : (k + 1) * 128,
            H + k * 128 : H + (k + 1) * 128,
            2 * H + k * 128 : 2 * H + (k + 1) * 128,
        ]
        WihT_aug = np.vstack([W_ih[rows, :].T, b_ih[rows][None, :]]).astype(BF)
        WhhT_aug = np.vstack([W_hh[rows, :].T, b_hh[rows][None, :]]).astype(BF)
        m = dict(shared)
        m.update(
            {
                "keys": keys_k.astype(BF),
                "keysT": keysT_aug,
                "wih": WihT_aug,
                "whh": WhhT_aug,
                "hmy0": np.ascontiguousarray(h0[k * 128 : (k + 1) * 128][None, :]),
            }
        )
        in_maps.append(m)

    nc = _get_nc()
    res = run_bass_kernel_spmd(nc, in_maps, core_ids=list(range(NCORES)))
    r = res.results
    outs = np.asarray(r[0]["outs"], f32).reshape(T, V)
    hfin = np.asarray(r[0]["hfin"], f32).reshape(H)
    Z = np.asarray(r[0]["zout"], f32).reshape(T)
    att = np.concatenate(
        [np.asarray(r[k]["attns"]).astype(f32).reshape(T, SS) for k in range(NCORES)],
        axis=1,
    )
    attns = att / Z[:, None]
    return outs[None], hfin[None, None], attns[None]


if __name__ == "__main__":
    _get_nc()
    print("build+compile OK")
